# revision 5
# baseline (speedup 1.0000x reference)
"""Trainium2 Bass kernel for nn_ACOPFEnforcer (per-node-type MLP, no message passing).

Math per node type t (sizes SB=4000, PQ=200000, PV=80000, NB=116000):
    inp = concat(x_t, c_t)                      # [N, 11]
    z_l = inp @ W[l,t] + b[l,t]                 # l = 0..2, [N, 128]
    h_l = ELU(z_l)
    P_l = h_l[:, :64].sum(-1); Q_l = h_l[:, 64:].sum(-1)
    out[l*N+n] = ELU(P_l[n]*W2[0] + Q_l[n]*W2[1] + b2)   # [128]
Output = concat over types of the [3*N_t, 128] blocks.

v2 design (replaces the Exp+combine pair with cheaper per-element work):
- stage-1 t1 = ELU(z)+1 computed as ONE custom DVE op per tile: a convex
  4-piece max-of-lines  max(z+1, a*z+b, g*z+d, 0)  with per-channel
  (a,b,d) constants per (type,layer) segment (z ~ N(0, sigma_ch), fits
  are sigma-parameterized and zero-mean-corrected so the P/Q sums don't
  accumulate bias).
- stage-2 out = ELU1(y) via two alternating paths (env K_W2 = % on PWL):
    PWL path (DVE): max(y+C0, C1*y+C3, C2*y+C3, 0), C0=beta+1 exact,
      fan line shares the C3 intercept, C2 global.
    exact path: ScalarE Exp(y+beta) and Relu(y+beta) then one GpSimd
      scalar_tensor_tensor  (e MIN 1) ADD r  == ELU1 exactly.
  This balances DVE (which otherwise does 2 ops/elem) against the
  otherwise idle Scalar and GpSimd engines.
- biases: b_fc rides a ones-row in the stage-1 matmul (K=12); the
  stage-2 shift beta = b2 - colsum(m2) lives in the op constants /
  activation bias.

Output is written bf16 in [channel, node] layout (values stored as ELU+1)
and transposed/cast/-1 on the host.
"""

import os
import numpy as np
import ml_dtypes

import concourse.bass as bass
import concourse.tile as tile
from concourse import bacc, mybir
from concourse.bass_utils import run_bass_kernel_spmd

BF16 = mybir.dt.bfloat16
F32 = mybir.dt.float32
AF = mybir.ActivationFunctionType
ALU = mybir.AluOpType

NODE_TYPES = ["SB", "PQ", "PV", "NB"]
SIZES = {"SB": 4000, "PQ": 200000, "PV": 80000, "NB": 116000}
NUM_LAYERS = 3
N_CORES = 8
TILE_N = 1024          # unit width
SUB = 512              # PSUM bank width in fp32 == matmul ISA width cap
DELAY = int(os.environ.get("K_D", "2"))   # units between stage-1 and stage-2
RXPCT = int(os.environ.get("K_RX", "40"))  # % of stage-2 relu ops on the DVE
IN_K = 12              # 11 features + 1 ones row (b_fc)

PPC = {t: -(-SIZES[t] // (N_CORES * SUB)) * SUB for t in NODE_TYPES}
VPC = {t: SIZES[t] // N_CORES for t in NODE_TYPES}      # valid nodes/core
COLS = NUM_LAYERS * sum(PPC.values())                   # stream cols/core
N_UNITS = -(-COLS // TILE_N)
N_SEGS = NUM_LAYERS * 4

_CACHE = {}


# --------------------------------------------------------------------------
# PWL fits (host-side, numpy)
# --------------------------------------------------------------------------

def _elu1(v):
    return np.where(v > 0, v + 1.0, np.exp(np.minimum(v, 0)))


def _wline(v, f, w):
    sw = w.sum()
    if sw <= 0 or len(v) < 8:
        return None
    mv = (w * v).sum() / sw
    mf = (w * f).sum() / sw
    cvv = (w * (v - mv) ** 2).sum() / sw
    cvf = (w * (v - mv) * (f - mf)).sum() / sw
    if cvv < 1e-12:
        return None
    a = cvf / cvv
    return a, mf - a * mv


def _fit_s1_sigma(sig, n_grid=1601, iters=24):
    """max(v+1, a v + b, g v + d, 0) ~ ELU1 over N(0, sig); zero-mean err."""
    v = np.linspace(-4.5 * sig, 4.5 * sig, n_grid)
    w = np.exp(-0.5 * (v / sig) ** 2)
    f = _elu1(v)
    t2, t3 = -1.1 * sig, -2.6 * sig
    a, b = np.exp(t2), np.exp(t2) * (1 - t2)
    g, d = np.exp(t3), np.exp(t3) * (1 - t3)
    for _ in range(iters):
        vals = np.stack([v + 1.0, a * v + b, g * v + d, np.zeros_like(v)])
        win = vals.argmax(0)
        r = _wline(v[win == 1], f[win == 1], w[win == 1])
        if r is not None:
            a, b = r
        r = _wline(v[win == 2], f[win == 2], w[win == 2])
        if r is not None:
            g, d = r
    for _ in range(8):
        vals = np.stack([v + 1.0, a * v + b, g * v + d, np.zeros_like(v)])
        win = vals.argmax(0)
        gerr = ((vals.max(0) - f) * w).sum() / w.sum()
        p2 = w[win == 1].sum() / w.sum()
        p3 = w[win == 2].sum() / w.sum()
        tot = p2 + p3
        if tot < 0.01:
            break
        b -= gerr * (p2 / tot)
        d -= gerr * (p3 / tot)
    return a, b, g, d


def _s1_table():
    if "s1tab" in _CACHE:
        return _CACHE["s1tab"]
    sgrid = np.geomspace(0.02, 1.2, 40)
    prm = np.array([_fit_s1_sigma(s) for s in sgrid])
    _CACHE["s1tab"] = (np.log(sgrid), prm)
    return _CACHE["s1tab"]


def _s1_params(sigmas):
    lg, prm = _s1_table()
    ls = np.log(np.clip(sigmas, np.exp(lg[0]), np.exp(lg[-1])))
    out = np.empty((len(sigmas), 4))
    for j in range(4):
        out[:, j] = np.interp(ls, lg, prm[:, j])
    return out                       # per-channel a, b, g, d


def _fit_s2(y_samp, beta, c2_candidates=(0.3, 0.5, 0.7), iters=12):
    """Per channel fit of max(v+beta+1, C1 v + C3, C2 v + C3, 0) to
    ELU1(v+beta), v = y_raw samples. Returns (C1, C3, C2, score)."""
    Ns, C = y_samp.shape
    best = None
    for C2 in c2_candidates:
        C1 = np.zeros(C)
        C3 = np.zeros(C)
        sse = 0.0
        for ch in range(C):
            yt = y_samp[:, ch]
            v = yt - beta[ch]
            f = _elu1(yt)
            nl = yt < 0.5                # nonlinear-exposed samples
            if nl.sum() < 32:
                # channel never (rarely) sees the elbow: park both lines
                # far below everything over the sample range
                vmax = np.abs(v).max() + 10.0
                C1[ch] = 0.0
                C3[ch] = -max(C2, 1.0) * vmax
                g = np.maximum(v + beta[ch] + 1.0, 0.0)
                sse += ((g - f) ** 2).mean()
                continue
            sd = max(yt.std(), 0.3)
            t2 = -1.2 * sd
            a = np.exp(t2)
            c = np.exp(t2) * (1 - t2) + a * beta[ch]
            ones = np.ones_like(v)
            for _ in range(iters):
                vals = np.stack([v + beta[ch] + 1.0, a * v + c, C2 * v + c,
                                 np.zeros_like(v)])
                win = vals.argmax(0)
                m1 = win == 1
                if m1.sum() >= 8:
                    r = _wline(v[m1], f[m1], ones[m1])
                    if r is not None:
                        a = r[0]
                m12 = (win == 1) | (win == 2)
                if m12.sum() >= 8:
                    slope = np.where(win[m12] == 1, a, C2)
                    c = (f[m12] - slope * v[m12]).mean()
            g = np.stack([v + beta[ch] + 1.0, a * v + c, C2 * v + c,
                          np.zeros_like(v)]).max(0)
            e = ((g - f) ** 2).mean()
            ge = np.maximum(v + beta[ch] + 1.0, 0.0)
            e0 = ((ge - f) ** 2).mean()
            if e0 <= e:                  # parking beats the fit
                vmax = np.abs(v).max() + 10.0
                C1[ch] = 0.0
                C3[ch] = -max(C2, 1.0) * vmax
                sse += e0
            else:
                C1[ch] = a
                C3[ch] = c
                sse += e
        score = np.sqrt(sse / C)
        if best is None or score < best[3]:
            best = (C1.copy(), C3.copy(), C2, score)
    return best


# --------------------------------------------------------------------------
# custom DVE ops
# --------------------------------------------------------------------------

def _register_ops():
    if "op_s1" in _CACHE:
        return
    import concourse.dve_ops as dve_ops
    from concourse.dve_spec import (Spec, Src0, Src1, C0, C1, C2, C3, One,
                                    Zero, maxx, minn, lower,
                                    _spill_c3_to_src1)
    from concourse.dve_uop import DveOpSpec

    def reg(name, body, ref):
        spec = Spec(body=_spill_c3_to_src1(body), reference=ref)
        if name not in dve_ops._SUB_OPCODE_FOR_NAME:
            dve_ops._SUB_OPCODE_FOR_NAME[name] = \
                max(dve_ops._SUB_OPCODE_FOR_NAME.values()) + 1
        assert dve_ops._SUB_OPCODE_FOR_NAME[name] < 0x20
        shas = {}
        for ver in ("v3", "v4"):
            s = DveOpSpec(name=name, opcode=dve_ops._SUB_OPCODE_FOR_NAME[name],
                          uops=lower(spec, ver=ver), rd1_en=True)
            shas[ver] = s.sha(ver)
        op = dve_ops.DveOp(name, spec, subdim=False, uops_sha=shas)
        if not any(o.name == name for o in dve_ops.OPS):
            dve_ops.OPS.append(op)
        dve_ops.CUSTOM_DVE_SPECS[name] = spec
        return op

    # stage-2 exact combine: out = max(in0 + s0, min(in1, 1))
    bc = maxx(Src0 + C0, minn(Src1, One))
    _CACHE["op_comb"] = reg(
        "ELU1_COMBINE_ANT", bc,
        lambda in0, in1, c0, c1, c2: np.maximum(
            np.asarray(in0, np.float32) + c0,
            np.minimum(np.asarray(in1, np.float32), 1.0)))

    # stage-1: max(v+1, C0*v+C1, C2*v+C3, 0)   (C3 rides in1, C2 = imm2)
    b1 = maxx(maxx(Src0 + One, Src0 * C0 + C1),
              maxx(Src0 * C2 + C3, Zero))
    _CACHE["op_s1"] = reg(
        "ELU1_PWL_S1_ANT", b1,
        lambda in0, in1, c0, c1, c2: np.maximum.reduce([
            np.asarray(in0, np.float32) + 1.0,
            c0 * np.asarray(in0, np.float32) + c1,
            c2 * np.asarray(in0, np.float32) + np.asarray(in1, np.float32),
            np.zeros_like(np.asarray(in0, np.float32))]))



# --------------------------------------------------------------------------
# device kernel
# --------------------------------------------------------------------------

def _segments():
    segs = []
    c = 0
    for ti, t in enumerate(NODE_TYPES):
        for l in range(NUM_LAYERS):
            segs.append((ti * NUM_LAYERS + l, c, c + PPC[t]))
            c += PPC[t]
    assert c == COLS
    return segs


def _build_nc():
    _register_ops()
    nc = bacc.Bacc("TRN2", target_bir_lowering=False, debug=False,
                   enable_asserts=False, num_devices=N_CORES)

    inp_ap = nc.dram_tensor("inp_cat", [IN_K, COLS], BF16,
                            kind="ExternalInput").ap()
    wseg_ap = nc.dram_tensor("wseg", [IN_K, N_SEGS * 128], BF16,
                             kind="ExternalInput").ap()
    m2_ap = nc.dram_tensor("m2", [128, 128], BF16, kind="ExternalInput").ap()
    # s1 PWL constants: a, b, d as [128, N_SEGS]
    s1a_ap = nc.dram_tensor("s1a", [128, N_SEGS], F32, kind="ExternalInput").ap()
    s1b_ap = nc.dram_tensor("s1b", [128, N_SEGS], F32, kind="ExternalInput").ap()
    s1d_ap = nc.dram_tensor("s1d", [128, N_SEGS], F32, kind="ExternalInput").ap()
    b2adj_ap = nc.dram_tensor("b2adj", [128, 1], F32, kind="ExternalInput").ap()
    b2adjp1_ap = nc.dram_tensor("b2adjp1", [128, 1], F32, kind="ExternalInput").ap()
    out_ap = nc.dram_tensor("out", [128, COLS], BF16,
                            kind="ExternalOutput").ap()

    with tile.TileContext(nc) as tc:
        _emit(tc, inp_ap, wseg_ap, m2_ap, s1a_ap, s1b_ap, s1d_ap,
              b2adj_ap, b2adjp1_ap, out_ap)
    nc.compile()
    return nc


def _emit(tc, inp_ap, wseg_ap, m2_ap, s1a_ap, s1b_ap, s1d_ap,
          b2adj_ap, b2adjp1_ap, out_ap):
    nc = tc.nc
    from contextlib import ExitStack
    ctx = ExitStack()
    with ctx:
        op_s1 = _CACHE["op_s1"]
        op_comb = _CACHE["op_comb"]
        g1 = _CACHE["g1"]          # per-seg imm2 for op_s1 (python floats)

        consts = ctx.enter_context(tc.tile_pool(name="consts", bufs=1))
        p_inp = ctx.enter_context(tc.tile_pool(name="inp", bufs=3))
        p_t1 = ctx.enter_context(tc.tile_pool(name="t1", bufs=DELAY + 3))
        p_e = ctx.enter_context(tc.tile_pool(name="e", bufs=3))
        p_r = ctx.enter_context(tc.tile_pool(name="r", bufs=3))
        p_ot = ctx.enter_context(tc.tile_pool(name="ot", bufs=4))
        ps_z = ctx.enter_context(tc.tile_pool(name="zps", bufs=2, space="PSUM"))
        ps_y = ctx.enter_context(tc.tile_pool(name="yps", bufs=2, space="PSUM"))

        wseg = consts.tile([IN_K, N_SEGS * 128], BF16, tag="wseg", name="wseg")
        nc.sync.dma_start(wseg[:], wseg_ap[:])
        m2 = consts.tile([128, 128], BF16, tag="m2", name="m2")
        nc.sync.dma_start(m2[:], m2_ap[:])
        s1a = consts.tile([128, N_SEGS], F32, tag="s1a", name="s1a")
        nc.sync.dma_start(s1a[:], s1a_ap[:])
        s1b = consts.tile([128, N_SEGS], F32, tag="s1b", name="s1b")
        nc.sync.dma_start(s1b[:], s1b_ap[:])
        s1d = consts.tile([128, N_SEGS], F32, tag="s1d", name="s1d")
        nc.sync.dma_start(s1d[:], s1d_ap[:])
        b2adj = consts.tile([128, 1], F32, tag="b2adj", name="b2adj")
        nc.sync.dma_start(b2adj[:], b2adj_ap[:])
        b2adjp1 = consts.tile([128, 1], F32, tag="b2adjp1", name="b2adjp1")
        nc.sync.dma_start(b2adjp1[:], b2adjp1_ap[:])

        segs = _segments()

        def seg_ranges(a, w):
            """[(seg, off_in_tile, width)] covering columns [a, a+w)."""
            out = []
            for s, lo, hi in segs:
                l = max(a, lo)
                r = min(a + w, hi)
                if l < r:
                    out.append((s, l - a, r - l))
            return out

        OB = 4
        t1_live = {}
        itile_cur = [None, -1]
        cnt = [0]

        for k in range(N_UNITS + DELAY):
            a = k * TILE_N
            lo_w = min(TILE_N, COLS - a) if k < N_UNITS else 0
            j = k - DELAY
            hi_w = min(TILE_N, COLS - j * TILE_N) if j >= 0 else 0

            if lo_w:
                z = ps_z.tile([128, TILE_N], F32, tag="zps", name="zps")
                kb = k // OB
                if itile_cur[1] != kb:
                    span = min(OB * TILE_N, COLS - kb * OB * TILE_N)
                    itile = p_inp.tile([IN_K, OB * TILE_N], BF16, tag="inp",
                                       name="inp")
                    nc.sync.dma_start(
                        itile[:, 0:span],
                        inp_ap[:, kb * OB * TILE_N:kb * OB * TILE_N + span])
                    itile_cur[0] = itile
                    itile_cur[1] = kb
                itile = itile_cur[0]
                ioff = a - kb * OB * TILE_N
                for c0 in range(a, a + lo_w, SUB):
                    srs = seg_ranges(c0, SUB)
                    s = srs[0][0]  # SUB=512 never straddles (PPC % 512 == 0)
                    assert len(srs) == 1
                    nc.tensor.matmul(
                        z[:, c0 - a:c0 - a + SUB],
                        lhsT=wseg[:, s * 128:(s + 1) * 128],
                        rhs=itile[:, ioff + c0 - a:ioff + c0 - a + SUB],
                        start=True, stop=True)
                # stage-1 PWL (split at segment boundaries)
                t1 = p_t1.tile([128, TILE_N], BF16, tag="t1", name="t1")
                for s, off, w in seg_ranges(a, lo_w):
                    nc.vector._custom_dve(
                        op_s1, out=t1[:, off:off + w], in0=z[:, off:off + w],
                        in1=s1d[:, s:s + 1], s0=s1a[:, s:s + 1],
                        s1=s1b[:, s:s + 1], imm2=g1[s])
                t1_live[k] = t1

            if hi_w:
                y = ps_y.tile([128, TILE_N], F32, tag="yps", name="yps")
                t1j = t1_live.pop(j)
                for c0 in range(0, hi_w, SUB):
                    nc.tensor.matmul(
                        y[:, c0:c0 + SUB],
                        lhsT=m2[:, :],
                        rhs=t1j[:, c0:c0 + SUB],
                        start=True, stop=True)
                ot = p_ot.tile([128, TILE_N], BF16, tag="ot", name="ot")
                cnt[0] += 1
                e = p_e.tile([128, TILE_N], BF16, tag="e", name="e")
                nc.scalar.activation(e[:, 0:hi_w], y[:, 0:hi_w],
                                     AF.Exp, bias=b2adj[:, 0:1])
                nc.vector._custom_dve(
                    op_comb, out=ot[:, 0:hi_w], in0=y[:, 0:hi_w],
                    in1=e[:, 0:hi_w], s0=b2adjp1[:, 0:1])
                nc.sync.dma_start(out_ap[:, j * TILE_N:j * TILE_N + hi_w],
                                  ot[:, 0:hi_w])


# --------------------------------------------------------------------------
# host prep / postprocess
# --------------------------------------------------------------------------

def _prep_inputs(x_SB, c_SB, x_PQ, c_PQ, x_PV, c_PV, x_NB, c_NB,
                 W_fc, b_fc, W2, b2):
    bf = ml_dtypes.bfloat16
    xs = {"SB": x_SB, "PQ": x_PQ, "PV": x_PV, "NB": x_NB}
    cs = {"SB": c_SB, "PQ": c_PQ, "PV": c_PV, "NB": c_NB}

    w2f = W2.astype(np.float32)
    m2 = np.zeros((128, 128), dtype=bf)
    m2[:64, :] = w2f[0][None, :].astype(bf)
    m2[64:, :] = w2f[1][None, :].astype(bf)
    m2f = m2.astype(np.float32)
    b2adj_v = b2.astype(np.float32) - m2f.sum(axis=0)

    # stage-1 weights: rows 0..10 = W, row 11 = b_fc (ones row in inputs)
    wseg = np.zeros((IN_K, N_SEGS * 128), dtype=bf)
    sig_seg = np.empty((N_SEGS, 128))
    for ti in range(4):
        for l in range(NUM_LAYERS):
            s = ti * NUM_LAYERS + l
            blk = np.zeros((IN_K, 128), dtype=np.float32)
            blk[:11] = W_fc[l, ti]
            blk[11] = b_fc[l, ti]
            wseg[:, s * 128:(s + 1) * 128] = blk.astype(bf)
            sig_seg[s] = np.linalg.norm(
                wseg[:11, s * 128:(s + 1) * 128].astype(np.float32), axis=0)

    # --- stage-1 PWL constants (sigma-interpolated fits) ---
    s1a = np.empty((128, N_SEGS), np.float32)
    s1b = np.empty((128, N_SEGS), np.float32)
    s1d = np.empty((128, N_SEGS), np.float32)
    g1 = []
    for s in range(N_SEGS):
        prm = _s1_params(sig_seg[s])           # [128, 4] a,b,g,d
        s1a[:, s] = prm[:, 0]
        s1b[:, s] = prm[:, 1]
        # imm2 (g) must be one float per call: use the weighted median g,
        # then refit d per channel with g fixed: d' = d + (g_ch - g)*v*...
        gmed = float(np.median(prm[:, 2]))
        for ch in range(128):
            a_, b_, g_, d_ = prm[ch]
            if abs(g_ - gmed) < 1e-9:
                s1d[ch, s] = d_
            else:
                # refit line3 intercept with slope gmed on N(0, sigma):
                # keep the tangency point v* = intersection-of-wins center.
                sig = sig_seg[s][ch]
                vg = np.linspace(-4.5 * sig, 4.5 * sig, 801)
                wg = np.exp(-0.5 * (vg / sig) ** 2)
                f = _elu1(vg)
                vals = np.stack([vg + 1.0, a_ * vg + b_, g_ * vg + d_,
                                 np.zeros_like(vg)])
                win = vals.argmax(0) == 2
                if win.sum() >= 8:
                    s1d[ch, s] = float(((f[win] - gmed * vg[win]) * wg[win]
                                        ).sum() / wg[win].sum())
                else:
                    s1d[ch, s] = d_ + 0.0
        g1.append(gmed)
    _CACHE["g1"] = g1

    b2adj = b2adj_v.reshape(128, 1).astype(np.float32)
    b2adjp1 = (b2adj_v + 1.0).reshape(128, 1).astype(np.float32)

    # --- concatenated per-core input stream (11 features + ones row) ---
    inp_cat = np.zeros((N_CORES, IN_K, COLS), dtype=bf)
    c = 0
    for t in NODE_TYPES:
        xT = xs[t].T.astype(bf)
        cT = cs[t].T.astype(bf)
        v = VPC[t]
        blk = np.zeros((N_CORES, IN_K, PPC[t]), dtype=bf)
        for i in range(N_CORES):
            blk[i, :4, :v] = xT[:, i * v:(i + 1) * v]
            blk[i, 4:11, :v] = cT[:, i * v:(i + 1) * v]
        blk[:, 11, :] = 1.0
        for l in range(NUM_LAYERS):
            inp_cat[:, :, c:c + PPC[t]] = blk
            c += PPC[t]
    assert c == COLS

    in_maps = []
    for i in range(N_CORES):
        in_maps.append(dict(inp_cat=inp_cat[i], wseg=wseg, m2=m2,
                            s1a=s1a, s1b=s1b, s1d=s1d,
                            b2adj=b2adj, b2adjp1=b2adjp1))
    return in_maps


def kernel(**inputs):
    in_maps = _prep_inputs(**inputs)   # also fills g1/g2 used by _build_nc
    if "nc" not in _CACHE:
        _CACHE["nc"] = _build_nc()
    nc = _CACHE["nc"]
    trace = bool(int(os.environ.get("K_TRACE", "0")))
    res = run_bass_kernel_spmd(nc, in_maps, core_ids=list(range(N_CORES)),
                               trace=trace)
    _CACHE["last_result"] = res
    outs = res.results if hasattr(res, "results") else res

    full = np.empty((NUM_LAYERS * sum(SIZES.values()), 128), dtype=np.float32)
    row = 0
    type_row0 = {}
    for t in NODE_TYPES:
        type_row0[t] = row
        row += NUM_LAYERS * SIZES[t]
    for i in range(N_CORES):
        o = np.asarray(outs[i]["out"])           # [128, COLS] bf16
        oT = o.T.astype(np.float32) - 1.0        # stored as ELU+1
        base = 0
        for t in NODE_TYPES:
            for l in range(NUM_LAYERS):
                src = base + l * PPC[t]
                dst = type_row0[t] + l * SIZES[t] + i * VPC[t]
                full[dst:dst + VPC[t]] = oT[src:src + VPC[t]]
            base += NUM_LAYERS * PPC[t]
    return full
